# revision 46
# baseline (speedup 1.0000x reference)
"""MoE layer (gate top-2 + 8 experts + shared expert + LayerNorm) on 8 TRN2 cores.

Strategy: data-parallel over tokens with SPARSE top-2 expert compute. The
host permutes tokens across the 8 cores (pure relayout, _plan_routing) so
per-(core,expert) load fits per-expert capacities caps[e] in {256, 288}
(288 for globally oversubscribed experts); the device still computes its own
routing. Per core (1024 tokens):

  1. Gate (fp32): logits -> top-2 masks; the sigmoid for the combine weights
     is deferred to after the gelu phase (avoids act-table thrash).
  2. Slot assignment ON DEVICE: per-expert token slots via an inclusive
     cumulative-sum of the expert mask (triangular-matrix matmul over the
     partition axis + sequential per-tile offsets).
  3. Dispatch: gather each expert's tokens with a matmul against the one-hot
     selection matrix SeT [tok, cap] (built by is_equal(iota, pos) * mask).
  4. Expert FFN on the gathered tokens only: GEMM1 and GEMM2 merged into one
     software-pipelined hc loop (G1(hc+1) is emitted before G2(hc) so the
     tensor engine never waits on the gelu); b2 is added during the Ye
     PSUM->SBUF copy from a broadcast tile.
  5. Combine: scatter-add weighted expert outputs back to token order with a
     matmul against Sw = SeT^T @ diag(combine_w)  (built on the tensor engine).
  6. Shared expert (dense, inside the gate loop) + LayerNorm; out is written
     bf16 and upcast on host.

This cuts tensor-engine work from ~140 GFLOP/core (dense all-expert) to
~60 GFLOP/core. Expert GEMMs run in bf16 with fp32 PSUM accumulation; the
gate runs fp32 (bf16 flips near-tie top-2 picks and fails the 2e-2 gate).
Tokens over capacity are dropped gracefully (one-hot never fires); the
planner leaves CAP_SLACK=4 headroom so in practice nothing drops.

Layout/perf notes (per core):
  - Weight streaming (128MB/iter) is split across BOTH HWDGE queues (W1 +
    x/out on the Activation queue via nc.scalar.dma_start, W2 on the SP
    queue): one queue saturates at ~213 GB/s, two reach ~400 GB/s.
  - Weight DMAs are batched 4 hc-tiles at a time (0.5-1MB per transfer);
    host pre-arranges W1/W2 so each transfer is contiguous.
  - GEMM1 (lhsT=W1 tile [128d,128h], rhs=XeT [128d, cap]) produces h in
    [h, slot] layout, exactly the lhsT layout GEMM2 needs.
  - Matmul moving operands are capped at 512 elements (ISA limit); GEMM2
    orders cc-outer/dh-inner so the dh pair reuses each stationary.
  - PSUM: 6 banks hold GEMM2's DH*CC accumulators across the merged loop;
    a 2-bank ring serves everything else.
"""

import numpy as np
import ml_dtypes

BF16 = ml_dtypes.bfloat16

# Problem shapes (hardcoded per contest contract).
B, S, D, E, H, DO = 4, 2048, 1024, 8, 4096, 1024
N_TOK = B * S
N_CORES = 8
P = 128
CAP = 320                 # per-expert token capacity per core (legacy uniform)
CAP_SLACK = 4             # host plans to CAP-CAP_SLACK kept tokens per (core,e)
NB = 512                  # PSUM bank width (fp32)
# Per-expert capacities for the balanced-permutation path: 288 for experts
# whose global top-2 count exceeds 8*(256-CAP_SLACK), else 256. The default
# matches the contest input (global counts [1971 1988 2045 2169 2147 1971
# 2012 2081]); kernel() recomputes from its actual input.
CAPS_DEFAULT = (256, 256, 288, 288, 288, 256, 256, 288)


def _expert_caps(flat, gate_W, n_cores=N_CORES, slack=CAP_SLACK):
    logits = flat @ gate_W
    srt = np.argsort(-logits, axis=1)
    G = np.bincount(srt[:, :2].ravel(), minlength=gate_W.shape[1])
    return tuple(288 if g > n_cores * (256 - slack) else 256 for g in G)


def _plan_routing(flat, gate_W, caps, n_cores=N_CORES, slack=CAP_SLACK):
    """Host-side token->core assignment balancing per-(core,expert) load.

    The device computes its own top-2 routing and assigns expert slots
    first-come-first-served in core token order with a fixed capacity of
    cap_dev per (core, expert). This planner permutes tokens across cores so
    every (core, expert) KEEP-count stays <= cap_dev - slack, and places any
    token whose expert contribution must be dropped (expert globally over
    8*(cap_dev-slack)) at the END of its core's order, choosing the
    lowest-combine-weight tokens as the drops. Pure relayout: the device
    still routes on its own; this only decides which tokens live on which
    core and in what order.
    """
    N, E_ = flat.shape[0], gate_W.shape[1]
    n_sh = N // n_cores
    cap_plan = np.asarray(caps, np.int64) - slack
    logits = flat @ gate_W
    srt = np.argsort(-logits, axis=1)
    e1, e2 = srt[:, 0], srt[:, 1]
    l1 = np.take_along_axis(logits, e1[:, None], 1)[:, 0]
    l2 = np.take_along_axis(logits, e2[:, None], 1)[:, 0]
    w2 = 1.0 / (1.0 + np.exp(np.minimum(l1 - l2, 50.0)))
    wts = np.stack([1.0 - w2, w2], 1)                  # [N, 2]
    exps = np.stack([e1, e2], 1)                       # [N, 2]

    # greedy balance: every token-expert pair counts toward its core's load
    cnt = np.zeros((n_cores, E_), np.int64)
    size = np.zeros(n_cores, np.int64)
    assign = np.full(N, -1, np.int64)
    rng = np.random.default_rng(0)
    for t in rng.permutation(N):
        ea, eb = int(exps[t, 0]), int(exps[t, 1])
        best, bkey = -1, None
        for c in range(n_cores):
            if size[c] >= n_sh:
                continue
            over = ((cnt[c, ea] >= cap_plan[ea]) +
                    (cnt[c, eb] >= cap_plan[eb]))
            load = max(int(cnt[c, ea]) - int(cap_plan[ea]),
                       int(cnt[c, eb]) - int(cap_plan[eb]))
            key = (over, load, int(size[c]))
            if bkey is None or key < bkey:
                best, bkey = c, key
        assign[t] = best
        size[best] += 1
        cnt[best, ea] += 1
        cnt[best, eb] += 1

    # per-core ordering: tokens carrying a low-weight contribution of an
    # over-capacity (core, expert) go last, so the device's FCFS slotting
    # drops exactly those.
    perm_parts = []
    caps_a = np.asarray(caps, np.int64)
    for c in range(n_cores):
        toks = np.where(assign == c)[0]
        over_e = set(np.where(cnt[c] > caps_a)[0].tolist())
        key = np.full(len(toks), np.inf)
        for k in range(2):
            sel = np.array([int(exps[t, k]) in over_e for t in toks])
            if sel.any():
                key[sel] = np.minimum(key[sel], wts[toks[sel], k])
        perm_parts.append(toks[np.argsort(-key, kind="stable")])
    return np.concatenate(perm_parts)


def _chunks(c):
    out = []
    o = 0
    while o < c:
        out.append((o, min(P, c - o)))
        o += P
    return out


def build_moe_bass(n_sh=N_TOK // N_CORES, d=D, e_=E, h_=H, do=DO, loop_n=1,
                   caps=CAPS_DEFAULT):
    """Build the single-core SPMD Bass program.

    loop_n > 1 wraps the whole compute body in an on-device For_i loop —
    used only for timing (amortizes the host dispatch overhead away).
    """
    import concourse.bass as bass
    import concourse.mybir as mybir
    import concourse.tile as tile
    from concourse import bacc

    f32 = mybir.dt.float32
    bf16 = mybir.dt.bfloat16
    AF = mybir.ActivationFunctionType
    ALU = mybir.AluOpType
    X = mybir.AxisListType.X

    DC = d // P        # d chunks
    HC = h_ // P       # h chunks
    TT = n_sh // P     # token tiles of 128
    DH = do // NB      # do chunks of 512
    CAPM = max(caps)   # padded tile capacity
    CCs = [_chunks(c) for c in caps]   # per-expert capacity chunks
    CCM = max(len(cc) for cc in CCs)

    HG = 4             # hc tiles per weight-DMA group
    HGN = HC // HG

    nc = bacc.Bacc("TRN2", target_bir_lowering=False)

    xT_t = nc.dram_tensor("xT", [TT, P, DC, P], f32, kind="ExternalInput")
    xtm_t = nc.dram_tensor("xtm", [P, TT, d], bf16, kind="ExternalInput")
    w1_t = nc.dram_tensor("w1h", [e_, HGN, P, HG, DC, P], bf16,
                          kind="ExternalInput")
    w2_t = nc.dram_tensor("w2h", [e_, HGN, P, HG, do], bf16,
                          kind="ExternalInput")
    gw_t = nc.dram_tensor("gwr", [P, DC, e_], f32, kind="ExternalInput")
    sw_t = nc.dram_tensor("swh", [P, DC, do], bf16, kind="ExternalInput")
    b1_t = nc.dram_tensor("b1h", [P, e_, HC], f32, kind="ExternalInput")
    b2_t = nc.dram_tensor("b2h", [e_, do], bf16, kind="ExternalInput")
    sb_t = nc.dram_tensor("sbh", [1, do], bf16, kind="ExternalInput")
    gam_t = nc.dram_tensor("gam", [do], f32, kind="ExternalInput")
    bet_t = nc.dram_tensor("bet", [do], f32, kind="ExternalInput")
    tri_t = nc.dram_tensor("tri", [P, P], bf16, kind="ExternalInput")
    idn_t = nc.dram_tensor("idn", [P, P], bf16, kind="ExternalInput")
    iot_t = nc.dram_tensor("iot", [CAPM], f32, kind="ExternalInput")
    out_t = nc.dram_tensor("out", [n_sh, do], bf16, kind="ExternalOutput")

    with tile.TileContext(nc) as tc:
        with (
            tc.tile_pool(name="resident", bufs=1) as resident,
            tc.tile_pool(name="xtp", bufs=2) as xtp,
            tc.tile_pool(name="xbp", bufs=2) as xbp,
            tc.tile_pool(name="setp", bufs=2) as setp,
            tc.tile_pool(name="xep", bufs=2) as xep,
            tc.tile_pool(name="hp", bufs=4) as hp,
            tc.tile_pool(name="yep", bufs=2) as yep,
            tc.tile_pool(name="swp", bufs=2) as swp,
            tc.tile_pool(name="dgp", bufs=2) as dgp,
            tc.tile_pool(name="b2p", bufs=2) as b2p,
            tc.tile_pool(name="w1p", bufs=3) as w1p,
            tc.tile_pool(name="w2p", bufs=3) as w2p,
            tc.tile_pool(name="gatep", bufs=2) as gatep,
            tc.tile_pool(name="lnp", bufs=3) as lnp,
            tc.tile_pool(name="outp", bufs=2) as outp,
            tc.tile_pool(name="psp", bufs=2, space="PSUM") as psp,
            tc.tile_pool(name="psB", bufs=DH * CCM, space="PSUM") as psB,
        ):
            def ps_tile(name):
                return psp.tile([P, NB], f32, tag="ps", name=name)

            # ---- resident loads ----
            gw = resident.tile([P, DC, e_], f32)
            nc.sync.dma_start(gw[:], gw_t[:])
            sw = resident.tile([P, DC, do], bf16)
            nc.sync.dma_start(sw[:], sw_t[:])
            b1 = resident.tile([P, e_, HC], f32)
            nc.sync.dma_start(b1[:], b1_t[:])
            sb = resident.tile([1, do], bf16)
            nc.sync.dma_start(sb[:], sb_t[:])
            tri = resident.tile([P, P], bf16)
            nc.sync.dma_start(tri[:], tri_t[:])
            idn = resident.tile([P, P], bf16)
            nc.sync.dma_start(idn[:], idn_t[:])
            xtm = resident.tile([P, TT, d], bf16)
            nc.scalar.dma_start(xtm[:], xtm_t[:])

            ones = resident.tile([1, P], bf16)
            nc.vector.memset(ones[:], 1.0)
            onef = resident.tile([1, P], f32)
            nc.vector.memset(onef[:], 1.0)
            onec = resident.tile([P, 1], bf16)
            nc.vector.memset(onec[:], 1.0)
            epst = resident.tile([P, 1], f32)
            nc.vector.memset(epst[:], 1e-5)

            # gamma/beta/iota broadcast across partitions via stride-0 DMA
            def bc_load(dst, src_ap):
                nc.gpsimd.dma_start(
                    out=dst,
                    in_=bass.AP(tensor=src_ap.tensor, offset=src_ap.offset,
                                ap=[[0, P]] + [list(a) for a in src_ap.ap]),
                )

            gam_bc = resident.tile([P, do], f32)
            bc_load(gam_bc[:], gam_t[:])
            bet_bc = resident.tile([P, do], f32)
            bc_load(bet_bc[:], bet_t[:])
            iota_bc = resident.tile([P, CAPM], f32)
            bc_load(iota_bc[:], iot_t[:])

            # accumulator [tok, do] fp32; combine weights / masks [tok, e]
            acc = resident.tile([P, TT, do], f32)
            comb = resident.tile([P, TT, e_], f32)
            mskf = resident.tile([P, TT, e_], f32)
            mskb = resident.tile([P, TT, e_], bf16)
            msk1a = resident.tile([P, TT, e_], f32)
            msk2a = resident.tile([P, TT, e_], f32)
            d21a = resident.tile([P, TT], f32)
            w2va = resident.tile([P, TT], f32)
            w1va = resident.tile([P, TT], f32)
            pos = resident.tile([P, TT, e_], f32)
            offr = resident.tile([1, TT, e_], f32)
            totr = resident.tile([1, TT, e_], f32)
            cum = resident.tile([P, TT, e_], f32)

            def _body():
                # ---- gate (fp32) + shared expert, one token-tile at a time --
                for t in range(TT):
                    xt_c = xtp.tile([P, DC, P], f32, tag="xt_c")
                    nc.sync.dma_start(xt_c[:], xT_t[t])
                    xb_c = xbp.tile([P, DC, P], bf16, tag="xb_c")
                    nc.vector.tensor_copy(xb_c[:], xt_c[:])
                    pg = ps_tile("pg")
                    for dc in range(DC):
                        nc.tensor.matmul(
                            pg[:, 0:e_], xt_c[:, dc, :], gw[:, dc, :],
                            start=(dc == 0), stop=(dc == DC - 1),
                        )
                    lg = gatep.tile([P, e_], f32, tag="lg")
                    nc.vector.tensor_copy(lg[:], pg[:, 0:e_])
                    m1 = gatep.tile([P, 1], f32, tag="m1")
                    nc.vector.reduce_max(m1[:], lg[:], axis=X)
                    mask1 = gatep.tile([P, e_], f32, tag="mask1")
                    nc.vector.tensor_scalar(mask1[:], lg[:], m1[:], None, ALU.is_ge)
                    l2 = gatep.tile([P, e_], f32, tag="l2")
                    nc.vector.scalar_tensor_tensor(
                        l2[:], in0=mask1[:], scalar=-1e30, in1=lg[:],
                        op0=ALU.mult, op1=ALU.add,
                    )
                    m2 = gatep.tile([P, 1], f32, tag="m2")
                    nc.vector.reduce_max(m2[:], l2[:], axis=X)
                    mask2 = gatep.tile([P, e_], f32, tag="mask2")
                    nc.vector.tensor_scalar(mask2[:], l2[:], m2[:], None, ALU.is_ge)
                    nc.vector.tensor_add(mskf[:, t, :], mask1[:], mask2[:])
                    nc.vector.tensor_copy(mskb[:, t, :], mskf[:, t, :])
                    nc.vector.tensor_sub(d21a[:, t:t + 1], m2[:], m1[:])
                    nc.vector.tensor_copy(msk1a[:, t, :], mask1[:])
                    nc.vector.tensor_copy(msk2a[:, t, :], mask2[:])

                    # shared expert: acc = 0.5*gelu(x @ shared_W + shared_b)
                    for dh in range(DH):
                        ps = ps_tile("sh")
                        nc.tensor.matmul(
                            ps[:], ones[0:1, :], sb[0:1, dh * NB:(dh + 1) * NB],
                            start=True, stop=False,
                        )
                        for dc in range(DC):
                            nc.tensor.matmul(
                                ps[:], xb_c[:, dc, :],
                                sw[:, dc, dh * NB:(dh + 1) * NB],
                                start=False, stop=(dc == DC - 1),
                            )
                        a_sl = acc[:, t, dh * NB:(dh + 1) * NB]
                        nc.scalar.activation(a_sl, ps[:], AF.Gelu)
                        nc.vector.tensor_scalar_mul(a_sl, a_sl, 0.5)

                # deferred gate weights: one sigmoid for all tiles (avoids
                # act-table thrash between Gelu and Sigmoid inside the loop)
                nc.scalar.activation(w2va[:], d21a[:], AF.Sigmoid)
                nc.vector.tensor_scalar(w1va[:], w2va[:], -1.0, 1.0,
                                        ALU.mult, ALU.add)
                for t in range(TT):
                    nc.vector.tensor_scalar_mul(msk1a[:, t, :], msk1a[:, t, :],
                                                w1va[:, t:t + 1])
                    nc.vector.scalar_tensor_tensor(
                        comb[:, t, :], in0=msk2a[:, t, :],
                        scalar=w2va[:, t:t + 1], in1=msk1a[:, t, :],
                        op0=ALU.mult, op1=ALU.add,
                    )

                # ---- slot assignment: pos = incl-cumsum(mask) - 1 ----------
                pc = ps_tile("pc")
                nc.tensor.matmul(pc[:, 0:TT * e_], tri[:], mskb[:],
                                 start=True, stop=True)
                nc.vector.tensor_copy(cum[:], pc[:, 0:TT * e_])
                pt = ps_tile("pt")
                nc.tensor.matmul(pt[0:1, 0:TT * e_], onec[:], mskb[:],
                                 start=True, stop=True)
                nc.vector.tensor_copy(totr[:], pt[0:1, 0:TT * e_])
                nc.vector.memset(offr[0:1, 0, :], 0.0)
                for t in range(1, TT):
                    nc.vector.tensor_add(offr[0:1, t, :], offr[0:1, t - 1, :],
                                         totr[0:1, t - 1, :])
                pb = ps_tile("pb")
                nc.tensor.matmul(pb[:, 0:TT * e_], onef[0:1, :], offr[:],
                                 start=True, stop=True)
                # pos = (cum - 1) + off_broadcast
                nc.vector.scalar_tensor_tensor(
                    pos[:], in0=cum[:], scalar=-1.0, in1=pb[:, 0:TT * e_],
                    op0=ALU.add, op1=ALU.add,
                )

                # ---- experts (sparse, capacity CAP) ------------------------
                for e in range(e_):
                    cap, CC = caps[e], CCs[e]
                    # SeT [tok, cap] one-hot, bf16
                    seT = setp.tile([P, TT, CAPM], bf16, tag="seT")
                    for t in range(TT):
                        nc.vector.tensor_scalar(seT[:, t, 0:cap],
                                                iota_bc[:, 0:cap],
                                                pos[:, t, e:e + 1],
                                                mskf[:, t, e:e + 1],
                                                ALU.is_equal, ALU.mult)

                    # dispatch: XeT[d, c] = sum_t x[t, d] * SeT[t, c]
                    xeT = xep.tile([P, DC, CAPM], bf16, tag="xeT")
                    for dc in range(DC):
                        pd = ps_tile("pd")
                        for tt in range(TT):
                            nc.tensor.matmul(
                                pd[:, 0:cap], xtm[:, tt, dc * P:(dc + 1) * P],
                                seT[:, tt, 0:cap],
                                start=(tt == 0), stop=(tt == TT - 1),
                            )
                        nc.vector.tensor_copy(xeT[:, dc, 0:cap], pd[:, 0:cap])

                    # Sw = SeT^T @ diag(comb_e)  [CAP, tok] bf16
                    sw_e = swp.tile([P, CCM, n_sh], bf16, tag="sw_e")
                    for t in range(TT):
                        dg = dgp.tile([P, P], bf16, tag="dg")
                        nc.vector.tensor_scalar_mul(dg[:], idn[:],
                                                    comb[:, t, e:e + 1])
                        for ci, (co, sz) in enumerate(CC):
                            pw = ps_tile("pw")
                            nc.tensor.matmul(pw[0:sz, 0:P],
                                             seT[:, t, co:co + sz], dg[:],
                                             start=True, stop=True)
                            nc.vector.tensor_copy(
                                sw_e[0:sz, ci, t * P:(t + 1) * P],
                                pw[0:sz, 0:P])

                    # GEMM1+GEMM2 merged, software-pipelined per hc:
                    #   G1(hc) -> gelu(hc) -> (next iter: G1(hc+1) emitted
                    #   before G2(hc) so PE never waits on the gelu).
                    # GEMM2's DH*CC psum tiles accumulate across all hc.
                    b2bc = b2p.tile([P, do], bf16, tag="b2bc")
                    bc_load(b2bc[:], b2_t[e])
                    ye = yep.tile([P, CCM, do], bf16, tag="ye")
                    p2s = [psB.tile([P, NB], f32, tag="p2", name=f"p2_{i}")
                           for i in range(DH * len(CC))]
                    h_tiles = {}
                    w2_tiles = {}

                    def emit_g1(hc):
                        g, j = hc // HG, hc % HG
                        if j == 0:
                            # W1 kicks ride the Activation HWDGE queue, W2 the
                            # SP queue: two queues ~doubles achieved HBM BW.
                            w1t = w1p.tile([P, HG, DC, P], bf16, tag="w1t")
                            nc.scalar.dma_start(w1t[:], w1_t[e, g])
                            emit_g1.w1t = w1t
                            w2t = w2p.tile([P, HG, do], bf16, tag="w2t")
                            nc.sync.dma_start(w2t[:], w2_t[e, g])
                            w2_tiles[g] = w2t
                        p1 = ps_tile("p1")
                        for dc in range(DC):
                            nc.tensor.matmul(
                                p1[:, 0:cap], emit_g1.w1t[:, j, dc, :],
                                xeT[:, dc, 0:cap],
                                start=(dc == 0), stop=(dc == DC - 1),
                            )
                        h_t = hp.tile([P, CAPM], bf16, tag="h_t")
                        nc.scalar.activation(
                            h_t[:, 0:cap], p1[:, 0:cap],
                            AF.Gelu, bias=b1[:, e, hc:hc + 1], scale=1.0,
                        )
                        h_tiles[hc] = h_t

                    def emit_g2(hc):
                        g, j = hc // HG, hc % HG
                        h_t = h_tiles.pop(hc)
                        w2t = w2_tiles[g]
                        # cc outer, dh inner: the dh pair shares one stationary
                        for ci, (co, sz) in enumerate(CC):
                            for dh in range(DH):
                                nc.tensor.matmul(
                                    p2s[ci * DH + dh][0:sz, :],
                                    h_t[0:P, co:co + sz],
                                    w2t[:, j, dh * NB:(dh + 1) * NB],
                                    start=(hc == 0), stop=(hc == HC - 1),
                                )

                    emit_g1(0)
                    for hc in range(1, HC):
                        emit_g1(hc)
                        emit_g2(hc - 1)
                    emit_g2(HC - 1)
                    for ci, (co, sz) in enumerate(CC):
                        for dh in range(DH):
                            nc.vector.tensor_add(
                                ye[0:sz, ci, dh * NB:(dh + 1) * NB],
                                p2s[ci * DH + dh][0:sz, :],
                                b2bc[0:sz, dh * NB:(dh + 1) * NB])

                    # scatter-add: acc += Sw^T @ Ye (dh pair shares lhsT);
                    # for the last expert, LayerNorm+store of tile t is
                    # emitted right after its scatter so the LN tail overlaps
                    # the remaining scatter matmuls.
                    n_sub = do // 512 if do % 512 == 0 and do > 512 else 1
                    for t in range(TT):
                        pss = [ps_tile(f"sc{dh}") for dh in range(DH)]
                        for ci, (co, sz) in enumerate(CC):
                            for dh in range(DH):
                                nc.tensor.matmul(
                                    pss[dh][:],
                                    sw_e[0:sz, ci, t * P:(t + 1) * P],
                                    ye[0:sz, ci, dh * NB:(dh + 1) * NB],
                                    start=(ci == 0), stop=(ci == len(CC) - 1),
                                )
                        for dh in range(DH):
                            a_sl = acc[:, t, dh * NB:(dh + 1) * NB]
                            nc.vector.tensor_add(a_sl, a_sl, pss[dh][:])
                        if e != e_ - 1:
                            continue
                        # ---- LayerNorm over do, then write out ----
                        a_t = acc[:, t, :]
                        st = lnp.tile([P, n_sub, 6], f32, tag="st")
                        a_view = a_t.rearrange("p (s d) -> p s d", s=n_sub)
                        for s in range(n_sub):
                            nc.vector.bn_stats(st[:, s, :], a_view[:, s, :])
                        mv = lnp.tile([P, 2], f32, tag="mv")
                        nc.vector.bn_aggr(mv[:], st[:])
                        rstd = lnp.tile([P, 1], f32, tag="rstd")
                        nc.scalar.activation(rstd[:], mv[:, 1:2], AF.Sqrt,
                                             bias=epst[:, 0:1], scale=1.0)
                        nc.vector.reciprocal(rstd[:], rstd[:])
                        o_t = outp.tile([P, do], f32, tag="o_t")
                        ob_t = outp.tile([P, do], bf16, tag="ob_t")
                        nc.vector.tensor_scalar_sub(o_t[:], a_t, mv[:, 0:1])
                        nc.vector.scalar_tensor_tensor(
                            o_t[:], in0=o_t[:], scalar=rstd[:], in1=gam_bc[:],
                            op0=ALU.mult, op1=ALU.mult,
                        )
                        nc.vector.tensor_add(ob_t[:], o_t[:], bet_bc[:])
                        nc.scalar.dma_start(out_t[t * P:(t + 1) * P, :], ob_t[:])

            if loop_n > 1:
                with tc.For_i(0, loop_n, 1):
                    _body()
            else:
                _body()

    nc.compile()
    return nc


def prep_inputs(x, W1, b1, W2, b2, gate_W, shared_W, shared_b, gamma, beta,
                n_cores=N_CORES, caps=CAPS_DEFAULT):
    """Host-side shard + relayout. Returns list of per-core in_maps."""
    n_tok = int(np.prod(x.shape[:-1]))
    d = x.shape[-1]
    e_, _, h_ = W1.shape
    do = W2.shape[-1]
    n_sh = n_tok // n_cores
    DC, HC, TT = d // P, h_ // P, n_sh // P

    HG = 4
    HGN = HC // HG
    DH = do // NB
    flat = np.ascontiguousarray(np.asarray(x, dtype=np.float32).reshape(n_tok, d))
    perm = _plan_routing(flat, np.asarray(gate_W, dtype=np.float32), caps,
                         n_cores)
    flat = np.ascontiguousarray(flat[perm])
    # weights: shared across cores (runtime copies per core)
    w1h = np.ascontiguousarray(
        np.asarray(W1, dtype=np.float32)
        .reshape(e_, DC, P, HGN, HG, P)
        .transpose(0, 3, 2, 4, 1, 5)                   # [e, g, p, j, dc, jj]
    ).astype(BF16)
    w2h = np.ascontiguousarray(
        np.asarray(W2, dtype=np.float32)
        .reshape(e_, HGN, HG, P, do)
        .transpose(0, 1, 3, 2, 4)                      # [e, g, p, j, do]
    ).astype(BF16)
    gwr = np.ascontiguousarray(
        np.asarray(gate_W, dtype=np.float32).reshape(DC, P, e_).transpose(1, 0, 2)
    )                                                  # [p, dc, e]
    swh = np.ascontiguousarray(
        np.asarray(shared_W, dtype=np.float32).reshape(DC, P, do).transpose(1, 0, 2)
    ).astype(BF16)                                     # [p, dc, do]
    b1h = np.ascontiguousarray(
        np.asarray(b1, dtype=np.float32).reshape(e_, HC, P).transpose(2, 0, 1)
    )                                                  # [p, e, hc]
    b2h = np.asarray(b2, dtype=np.float32).reshape(e_, do).astype(BF16)
    sbh = np.asarray(shared_b, dtype=np.float32).reshape(1, do).astype(BF16)
    gam = np.asarray(gamma, dtype=np.float32).reshape(do)
    bet = np.asarray(beta, dtype=np.float32).reshape(do)
    trih = np.triu(np.ones((P, P), np.float32)).astype(BF16)
    idnh = np.eye(P, dtype=np.float32).astype(BF16)
    ioth = np.arange(max(caps), dtype=np.float32)

    in_maps = []
    for c in range(n_cores):
        shard = flat[c * n_sh:(c + 1) * n_sh]          # [n_sh, d]
        xT = np.ascontiguousarray(
            shard.T.reshape(DC, P, TT, P).transpose(2, 1, 0, 3)
        )                                              # [t, p, dc, j]
        xtm = np.ascontiguousarray(
            shard.reshape(TT, P, d).transpose(1, 0, 2)
        ).astype(BF16)                                 # [p, tile, d]
        in_maps.append({
            "xT": xT, "xtm": xtm, "w1h": w1h, "w2h": w2h, "gwr": gwr,
            "swh": swh, "b1h": b1h, "b2h": b2h, "sbh": sbh, "gam": gam,
            "bet": bet, "tri": trih, "idn": idnh, "iot": ioth,
        })
    return in_maps, perm


_NC_CACHE = {}


def kernel(x, W1, b1, W2, b2, gate_W, shared_W, shared_b, gamma, beta):
    from concourse.bass_utils import run_bass_kernel_spmd

    n_tok = int(np.prod(x.shape[:-1]))
    n_sh = n_tok // N_CORES
    flat = np.asarray(x, dtype=np.float32).reshape(n_tok, x.shape[-1])
    caps = _expert_caps(flat, np.asarray(gate_W, dtype=np.float32))
    key = (n_sh, x.shape[-1], caps)
    if key not in _NC_CACHE:
        _NC_CACHE[key] = build_moe_bass(n_sh=n_sh, d=x.shape[-1],
                                        e_=W1.shape[0], h_=W1.shape[2],
                                        do=W2.shape[-1], caps=caps)
    nc = _NC_CACHE[key]
    in_maps, perm = prep_inputs(x, W1, b1, W2, b2, gate_W, shared_W, shared_b,
                                gamma, beta, caps=caps)
    res = run_bass_kernel_spmd(nc, in_maps, core_ids=list(range(N_CORES)))
    outs = [r["out"] for r in res.results]
    full = np.concatenate(outs, axis=0)               # [n_tok, do] (permuted)
    unperm = np.empty_like(full)
    unperm[perm] = full
    return unperm.reshape(*x.shape[:-1], full.shape[-1]).astype(np.float32)


# revision 50
# speedup vs baseline: 1.0326x; 1.0326x over previous
"""MoE layer (gate top-2 + 8 experts + shared expert + LayerNorm) on 8 TRN2 cores.

Strategy: data-parallel over tokens with SPARSE top-2 expert compute. The
host permutes tokens across the 8 cores (pure relayout, _plan_routing) so
per-(core,expert) load fits per-expert capacities caps[e] in {256, 288}
(288 for globally oversubscribed experts); the device still computes its own
routing. Per core (1024 tokens):

  1. Gate (fp32): logits -> top-2 masks; the sigmoid for the combine weights
     is deferred to after the gelu phase (avoids act-table thrash).
  2. Slot assignment ON DEVICE: per-expert token slots via an inclusive
     cumulative-sum of the expert mask (triangular-matrix matmul over the
     partition axis + sequential per-tile offsets).
  3. Dispatch: gather each expert's tokens with a matmul against the one-hot
     selection matrix SeT [tok, cap] (built by is_equal(iota, pos) * mask).
  4. Expert FFN on the gathered tokens only: GEMM1 and GEMM2 merged into one
     software-pipelined hc loop (G1(hc+1) is emitted before G2(hc) so the
     tensor engine never waits on the gelu); b2 is added during the Ye
     PSUM->SBUF copy from a broadcast tile.
  5. Combine: scatter-add weighted expert outputs back to token order with a
     matmul against Sw = SeT^T @ diag(combine_w)  (built on the tensor engine).
  6. Shared expert (dense, inside the gate loop) + LayerNorm; out is written
     bf16 and upcast on host.

This cuts tensor-engine work from ~140 GFLOP/core (dense all-expert) to
~60 GFLOP/core. Expert GEMMs run in bf16 with fp32 PSUM accumulation; the
gate runs fp32 (bf16 flips near-tie top-2 picks and fails the 2e-2 gate).
Tokens over capacity are dropped gracefully (one-hot never fires); the
planner leaves CAP_SLACK=4 headroom so in practice nothing drops.

Layout/perf notes (per core):
  - Weight streaming (128MB/iter) is split across BOTH HWDGE queues (W1 +
    x/out on the Activation queue via nc.scalar.dma_start, W2 on the SP
    queue): one queue saturates at ~213 GB/s, two reach ~400 GB/s.
  - Weight DMAs are batched 4 hc-tiles at a time (0.5-1MB per transfer);
    host pre-arranges W1/W2 so each transfer is contiguous.
  - GEMM1 (lhsT=W1 tile [128d,128h], rhs=XeT [128d, cap]) produces h in
    [h, slot] layout, exactly the lhsT layout GEMM2 needs.
  - Matmul moving operands are capped at 512 elements (ISA limit); GEMM2
    orders cc-outer/dh-inner so the dh pair reuses each stationary.
  - PSUM: 6 banks hold GEMM2's DH*CC accumulators across the merged loop;
    a 2-bank ring serves everything else.
"""

import numpy as np
import ml_dtypes

BF16 = ml_dtypes.bfloat16

# Problem shapes (hardcoded per contest contract).
B, S, D, E, H, DO = 4, 2048, 1024, 8, 4096, 1024
N_TOK = B * S
N_CORES = 8
P = 128
CAP = 320                 # per-expert token capacity per core (legacy uniform)
CAP_SLACK = 4             # host plans to CAP-CAP_SLACK kept tokens per (core,e)
NB = 512                  # PSUM bank width (fp32)
# Per-expert capacities for the balanced-permutation path: 288 for experts
# whose global top-2 count exceeds 8*(256-CAP_SLACK), else 256. The default
# matches the contest input (global counts [1971 1988 2045 2169 2147 1971
# 2012 2081]); kernel() recomputes from its actual input.
CAPS_DEFAULT = (256, 256, 288, 288, 288, 256, 256, 288)


def _expert_caps(flat, gate_W, n_cores=N_CORES, slack=CAP_SLACK):
    logits = flat @ gate_W
    srt = np.argsort(-logits, axis=1)
    G = np.bincount(srt[:, :2].ravel(), minlength=gate_W.shape[1])
    return tuple(288 if g > n_cores * (256 - slack) else 256 for g in G)


def _plan_routing(flat, gate_W, caps, n_cores=N_CORES, slack=CAP_SLACK):
    """Host-side token->core assignment balancing per-(core,expert) load.

    The device computes its own top-2 routing and assigns expert slots
    first-come-first-served in core token order with a fixed capacity of
    cap_dev per (core, expert). This planner permutes tokens across cores so
    every (core, expert) KEEP-count stays <= cap_dev - slack, and places any
    token whose expert contribution must be dropped (expert globally over
    8*(cap_dev-slack)) at the END of its core's order, choosing the
    lowest-combine-weight tokens as the drops. Pure relayout: the device
    still routes on its own; this only decides which tokens live on which
    core and in what order.
    """
    N, E_ = flat.shape[0], gate_W.shape[1]
    n_sh = N // n_cores
    cap_plan = np.asarray(caps, np.int64) - slack
    logits = flat @ gate_W
    srt = np.argsort(-logits, axis=1)
    e1, e2 = srt[:, 0], srt[:, 1]
    l1 = np.take_along_axis(logits, e1[:, None], 1)[:, 0]
    l2 = np.take_along_axis(logits, e2[:, None], 1)[:, 0]
    w2 = 1.0 / (1.0 + np.exp(np.minimum(l1 - l2, 50.0)))
    wts = np.stack([1.0 - w2, w2], 1)                  # [N, 2]
    exps = np.stack([e1, e2], 1)                       # [N, 2]

    # greedy balance: every token-expert pair counts toward its core's load
    cnt = np.zeros((n_cores, E_), np.int64)
    size = np.zeros(n_cores, np.int64)
    assign = np.full(N, -1, np.int64)
    rng = np.random.default_rng(0)
    for t in rng.permutation(N):
        ea, eb = int(exps[t, 0]), int(exps[t, 1])
        best, bkey = -1, None
        for c in range(n_cores):
            if size[c] >= n_sh:
                continue
            over = ((cnt[c, ea] >= cap_plan[ea]) +
                    (cnt[c, eb] >= cap_plan[eb]))
            load = max(int(cnt[c, ea]) - int(cap_plan[ea]),
                       int(cnt[c, eb]) - int(cap_plan[eb]))
            key = (over, load, int(size[c]))
            if bkey is None or key < bkey:
                best, bkey = c, key
        assign[t] = best
        size[best] += 1
        cnt[best, ea] += 1
        cnt[best, eb] += 1

    # per-core ordering: tokens carrying a low-weight contribution of an
    # over-capacity (core, expert) go last, so the device's FCFS slotting
    # drops exactly those.
    perm_parts = []
    caps_a = np.asarray(caps, np.int64)
    for c in range(n_cores):
        toks = np.where(assign == c)[0]
        over_e = set(np.where(cnt[c] > caps_a)[0].tolist())
        key = np.full(len(toks), np.inf)
        for k in range(2):
            sel = np.array([int(exps[t, k]) in over_e for t in toks])
            if sel.any():
                key[sel] = np.minimum(key[sel], wts[toks[sel], k])
        perm_parts.append(toks[np.argsort(-key, kind="stable")])
    return np.concatenate(perm_parts)


def _chunks(c):
    out = []
    o = 0
    while o < c:
        out.append((o, min(P, c - o)))
        o += P
    return out


def build_moe_bass(n_sh=N_TOK // N_CORES, d=D, e_=E, h_=H, do=DO, loop_n=1,
                   caps=CAPS_DEFAULT):
    """Build the single-core SPMD Bass program.

    loop_n > 1 wraps the whole compute body in an on-device For_i loop —
    used only for timing (amortizes the host dispatch overhead away).
    """
    import concourse.bass as bass
    import concourse.mybir as mybir
    import concourse.tile as tile
    from concourse import bacc

    f32 = mybir.dt.float32
    bf16 = mybir.dt.bfloat16
    AF = mybir.ActivationFunctionType
    ALU = mybir.AluOpType
    X = mybir.AxisListType.X

    DC = d // P        # d chunks
    HC = h_ // P       # h chunks
    TT = n_sh // P     # token tiles of 128
    DH = do // NB      # do chunks of 512
    CAPM = max(caps)   # padded tile capacity
    CCs = [_chunks(c) for c in caps]   # per-expert capacity chunks
    CCM = max(len(cc) for cc in CCs)

    HG = 4             # hc tiles per weight-DMA group
    HGN = HC // HG

    nc = bacc.Bacc("TRN2", target_bir_lowering=False)

    xT_t = nc.dram_tensor("xT", [TT, P, DC, P], f32, kind="ExternalInput")
    xtm_t = nc.dram_tensor("xtm", [P, TT, d], bf16, kind="ExternalInput")
    w1_t = nc.dram_tensor("w1h", [e_, HGN, P, HG, DC, P], bf16,
                          kind="ExternalInput")
    w2_t = nc.dram_tensor("w2h", [e_, HGN, P, HG, do], bf16,
                          kind="ExternalInput")
    gw_t = nc.dram_tensor("gwr", [P, DC, e_], f32, kind="ExternalInput")
    sw_t = nc.dram_tensor("swh", [P, DC, do], bf16, kind="ExternalInput")
    b1_t = nc.dram_tensor("b1h", [P, e_, HC], f32, kind="ExternalInput")
    b2_t = nc.dram_tensor("b2h", [e_, do], bf16, kind="ExternalInput")
    sb_t = nc.dram_tensor("sbh", [1, do], bf16, kind="ExternalInput")
    gam_t = nc.dram_tensor("gam", [do], f32, kind="ExternalInput")
    bet_t = nc.dram_tensor("bet", [do], f32, kind="ExternalInput")
    tri_t = nc.dram_tensor("tri", [P, P], bf16, kind="ExternalInput")
    idn_t = nc.dram_tensor("idn", [P, P], bf16, kind="ExternalInput")
    iot_t = nc.dram_tensor("iot", [CAPM], f32, kind="ExternalInput")
    out_t = nc.dram_tensor("out", [n_sh, do], bf16, kind="ExternalOutput")

    with tile.TileContext(nc) as tc:
        with (
            tc.tile_pool(name="resident", bufs=1) as resident,
            tc.tile_pool(name="xtp", bufs=2) as xtp,
            tc.tile_pool(name="xbp", bufs=2) as xbp,
            tc.tile_pool(name="setp", bufs=2) as setp,
            tc.tile_pool(name="xep", bufs=2) as xep,
            tc.tile_pool(name="hp", bufs=4) as hp,
            tc.tile_pool(name="yep", bufs=2) as yep,
            tc.tile_pool(name="swp", bufs=2) as swp,
            tc.tile_pool(name="dgp", bufs=2) as dgp,
            tc.tile_pool(name="b2p", bufs=2) as b2p,
            tc.tile_pool(name="w1p", bufs=3) as w1p,
            tc.tile_pool(name="w2p", bufs=3) as w2p,
            tc.tile_pool(name="gatep", bufs=2) as gatep,
            tc.tile_pool(name="lnp", bufs=3) as lnp,
            tc.tile_pool(name="outp", bufs=2) as outp,
            tc.tile_pool(name="psp", bufs=2, space="PSUM") as psp,
            tc.tile_pool(name="psB", bufs=DH * CCM, space="PSUM") as psB,
        ):
            def ps_tile(name):
                return psp.tile([P, NB], f32, tag="ps", name=name)

            # ---- resident loads ----
            gw = resident.tile([P, DC, e_], f32)
            nc.sync.dma_start(gw[:], gw_t[:])
            sw = resident.tile([P, DC, do], bf16)
            nc.sync.dma_start(sw[:], sw_t[:])
            b1 = resident.tile([P, e_, HC], f32)
            nc.sync.dma_start(b1[:], b1_t[:])
            sb = resident.tile([1, do], bf16)
            nc.sync.dma_start(sb[:], sb_t[:])
            tri = resident.tile([P, P], bf16)
            nc.sync.dma_start(tri[:], tri_t[:])
            idn = resident.tile([P, P], bf16)
            nc.sync.dma_start(idn[:], idn_t[:])
            xtm = resident.tile([P, TT, d], bf16)
            nc.scalar.dma_start(xtm[:], xtm_t[:])

            ones = resident.tile([1, P], bf16)
            nc.vector.memset(ones[:], 1.0)
            onef = resident.tile([1, P], f32)
            nc.vector.memset(onef[:], 1.0)
            onec = resident.tile([P, 1], bf16)
            nc.vector.memset(onec[:], 1.0)
            epst = resident.tile([P, 1], f32)
            nc.vector.memset(epst[:], 1e-5)

            # gamma/beta/iota broadcast across partitions via stride-0 DMA
            def bc_load(dst, src_ap):
                nc.gpsimd.dma_start(
                    out=dst,
                    in_=bass.AP(tensor=src_ap.tensor, offset=src_ap.offset,
                                ap=[[0, P]] + [list(a) for a in src_ap.ap]),
                )

            gam_bc = resident.tile([P, do], f32)
            bc_load(gam_bc[:], gam_t[:])
            bet_bc = resident.tile([P, do], f32)
            bc_load(bet_bc[:], bet_t[:])
            iota_bc = resident.tile([P, CAPM], f32)
            bc_load(iota_bc[:], iot_t[:])

            # accumulator [tok, do] fp32; combine weights / masks [tok, e]
            acc = resident.tile([P, TT, do], f32)
            comb = resident.tile([P, TT, e_], f32)
            mskf = resident.tile([P, TT, e_], f32)
            mskb = resident.tile([P, TT, e_], bf16)
            msk1a = resident.tile([P, TT, e_], f32)
            msk2a = resident.tile([P, TT, e_], f32)
            d21a = resident.tile([P, TT], f32)
            w2va = resident.tile([P, TT], f32)
            w1va = resident.tile([P, TT], f32)
            pos = resident.tile([P, TT, e_], f32)
            offr = resident.tile([1, TT, e_], f32)
            totr = resident.tile([1, TT, e_], f32)
            cum = resident.tile([P, TT, e_], f32)

            def _body():
                # ---- gate (fp32) + shared expert, one token-tile at a time --
                for t in range(TT):
                    xt_c = xtp.tile([P, DC, P], f32, tag="xt_c")
                    nc.sync.dma_start(xt_c[:], xT_t[t])
                    xb_c = xbp.tile([P, DC, P], bf16, tag="xb_c")
                    nc.vector.tensor_copy(xb_c[:], xt_c[:])
                    pg = ps_tile("pg")
                    for dc in range(DC):
                        nc.tensor.matmul(
                            pg[:, 0:e_], xt_c[:, dc, :], gw[:, dc, :],
                            start=(dc == 0), stop=(dc == DC - 1),
                        )
                    lg = gatep.tile([P, e_], f32, tag="lg")
                    nc.vector.tensor_copy(lg[:], pg[:, 0:e_])
                    m1 = gatep.tile([P, 1], f32, tag="m1")
                    nc.vector.reduce_max(m1[:], lg[:], axis=X)
                    mask1 = gatep.tile([P, e_], f32, tag="mask1")
                    nc.vector.tensor_scalar(mask1[:], lg[:], m1[:], None, ALU.is_ge)
                    l2 = gatep.tile([P, e_], f32, tag="l2")
                    nc.vector.scalar_tensor_tensor(
                        l2[:], in0=mask1[:], scalar=-1e30, in1=lg[:],
                        op0=ALU.mult, op1=ALU.add,
                    )
                    m2 = gatep.tile([P, 1], f32, tag="m2")
                    nc.vector.reduce_max(m2[:], l2[:], axis=X)
                    mask2 = gatep.tile([P, e_], f32, tag="mask2")
                    nc.vector.tensor_scalar(mask2[:], l2[:], m2[:], None, ALU.is_ge)
                    nc.vector.tensor_add(mskf[:, t, :], mask1[:], mask2[:])
                    nc.vector.tensor_copy(mskb[:, t, :], mskf[:, t, :])
                    nc.vector.tensor_sub(d21a[:, t:t + 1], m2[:], m1[:])
                    nc.vector.tensor_copy(msk1a[:, t, :], mask1[:])
                    nc.vector.tensor_copy(msk2a[:, t, :], mask2[:])

                    # shared expert: acc = 0.5*gelu(x @ shared_W + shared_b)
                    for dh in range(DH):
                        ps = ps_tile("sh")
                        nc.tensor.matmul(
                            ps[:], ones[0:1, :], sb[0:1, dh * NB:(dh + 1) * NB],
                            start=True, stop=False,
                        )
                        for dc in range(DC):
                            nc.tensor.matmul(
                                ps[:], xb_c[:, dc, :],
                                sw[:, dc, dh * NB:(dh + 1) * NB],
                                start=False, stop=(dc == DC - 1),
                            )
                        a_sl = acc[:, t, dh * NB:(dh + 1) * NB]
                        nc.scalar.activation(a_sl, ps[:], AF.Gelu)
                        nc.vector.tensor_scalar_mul(a_sl, a_sl, 0.5)

                # deferred gate weights: one sigmoid for all tiles (avoids
                # act-table thrash between Gelu and Sigmoid inside the loop)
                nc.scalar.activation(w2va[:], d21a[:], AF.Sigmoid)
                nc.vector.tensor_scalar(w1va[:], w2va[:], -1.0, 1.0,
                                        ALU.mult, ALU.add)
                for t in range(TT):
                    nc.vector.tensor_scalar_mul(msk1a[:, t, :], msk1a[:, t, :],
                                                w1va[:, t:t + 1])
                    nc.vector.scalar_tensor_tensor(
                        comb[:, t, :], in0=msk2a[:, t, :],
                        scalar=w2va[:, t:t + 1], in1=msk1a[:, t, :],
                        op0=ALU.mult, op1=ALU.add,
                    )

                # ---- slot assignment: pos = incl-cumsum(mask) - 1 ----------
                pc = ps_tile("pc")
                nc.tensor.matmul(pc[:, 0:TT * e_], tri[:], mskb[:],
                                 start=True, stop=True)
                nc.vector.tensor_copy(cum[:], pc[:, 0:TT * e_])
                pt = ps_tile("pt")
                nc.tensor.matmul(pt[0:1, 0:TT * e_], onec[:], mskb[:],
                                 start=True, stop=True)
                nc.vector.tensor_copy(totr[:], pt[0:1, 0:TT * e_])
                nc.vector.memset(offr[0:1, 0, :], 0.0)
                for t in range(1, TT):
                    nc.vector.tensor_add(offr[0:1, t, :], offr[0:1, t - 1, :],
                                         totr[0:1, t - 1, :])
                pb = ps_tile("pb")
                nc.tensor.matmul(pb[:, 0:TT * e_], onef[0:1, :], offr[:],
                                 start=True, stop=True)
                # pos = (cum - 1) + off_broadcast
                nc.vector.scalar_tensor_tensor(
                    pos[:], in0=cum[:], scalar=-1.0, in1=pb[:, 0:TT * e_],
                    op0=ALU.add, op1=ALU.add,
                )

                # ---- experts (sparse, capacity CAP) ------------------------
                for e in range(e_):
                    cap, CC = caps[e], CCs[e]
                    # SeT [tok, cap] one-hot, bf16
                    seT = setp.tile([P, TT, CAPM], bf16, tag="seT")
                    for t in range(TT):
                        nc.vector.tensor_scalar(seT[:, t, 0:cap],
                                                iota_bc[:, 0:cap],
                                                pos[:, t, e:e + 1],
                                                mskf[:, t, e:e + 1],
                                                ALU.is_equal, ALU.mult)

                    # dispatch: XeT[d, c] = sum_t x[t, d] * SeT[t, c]
                    xeT = xep.tile([P, DC, CAPM], bf16, tag="xeT")
                    for dc in range(DC):
                        pd = ps_tile("pd")
                        for tt in range(TT):
                            nc.tensor.matmul(
                                pd[:, 0:cap], xtm[:, tt, dc * P:(dc + 1) * P],
                                seT[:, tt, 0:cap],
                                start=(tt == 0), stop=(tt == TT - 1),
                            )
                        nc.vector.tensor_copy(xeT[:, dc, 0:cap], pd[:, 0:cap])

                    # Sw = SeT^T @ diag(comb_e)  [CAP, tok] bf16
                    sw_e = swp.tile([P, CCM, n_sh], bf16, tag="sw_e")
                    for t in range(TT):
                        dg = dgp.tile([P, P], bf16, tag="dg")
                        nc.vector.tensor_scalar_mul(dg[:], idn[:],
                                                    comb[:, t, e:e + 1])
                        for ci, (co, sz) in enumerate(CC):
                            pw = ps_tile("pw")
                            nc.tensor.matmul(pw[0:sz, 0:P],
                                             seT[:, t, co:co + sz], dg[:],
                                             start=True, stop=True)
                            nc.vector.tensor_copy(
                                sw_e[0:sz, ci, t * P:(t + 1) * P],
                                pw[0:sz, 0:P])

                    # GEMM1+GEMM2 merged, software-pipelined per hc:
                    #   G1(hc) -> gelu(hc) -> (next iter: G1(hc+1) emitted
                    #   before G2(hc) so PE never waits on the gelu).
                    # GEMM2's DH*CC psum tiles accumulate across all hc.
                    b2bc = b2p.tile([P, do], bf16, tag="b2bc")
                    bc_load(b2bc[:], b2_t[e])
                    ye = yep.tile([P, CCM, do], bf16, tag="ye")
                    p2s = [psB.tile([P, NB], f32, tag="p2", name=f"p2_{i}")
                           for i in range(DH * len(CC))]
                    h_tiles = {}
                    w2_tiles = {}

                    def emit_g1(hc):
                        g, j = hc // HG, hc % HG
                        if j == 0:
                            # W1 kicks ride the Activation HWDGE queue, W2 the
                            # SP queue: two queues ~doubles achieved HBM BW.
                            w1t = w1p.tile([P, HG, DC, P], bf16, tag="w1t")
                            nc.scalar.dma_start(w1t[:], w1_t[e, g])
                            emit_g1.w1t = w1t
                            w2t = w2p.tile([P, HG, do], bf16, tag="w2t")
                            nc.sync.dma_start(w2t[:], w2_t[e, g])
                            w2_tiles[g] = w2t
                        p1 = ps_tile("p1")
                        for dc in range(DC):
                            nc.tensor.matmul(
                                p1[:, 0:cap], emit_g1.w1t[:, j, dc, :],
                                xeT[:, dc, 0:cap],
                                start=(dc == 0), stop=(dc == DC - 1),
                            )
                        h_t = hp.tile([P, CAPM], bf16, tag="h_t")
                        nc.scalar.activation(
                            h_t[:, 0:cap], p1[:, 0:cap],
                            AF.Gelu, bias=b1[:, e, hc:hc + 1], scale=1.0,
                        )
                        h_tiles[hc] = h_t

                    def emit_g2(hc):
                        g, j = hc // HG, hc % HG
                        h_t = h_tiles.pop(hc)
                        w2t = w2_tiles[g]
                        # cc outer, dh inner: the dh pair shares one stationary
                        for ci, (co, sz) in enumerate(CC):
                            for dh in range(DH):
                                nc.tensor.matmul(
                                    p2s[ci * DH + dh][0:sz, :],
                                    h_t[0:P, co:co + sz],
                                    w2t[:, j, dh * NB:(dh + 1) * NB],
                                    start=(hc == 0), stop=(hc == HC - 1),
                                )

                    emit_g1(0)
                    for hc in range(1, HC):
                        emit_g1(hc)
                        emit_g2(hc - 1)
                    emit_g2(HC - 1)
                    for ci, (co, sz) in enumerate(CC):
                        for dh in range(DH):
                            nc.vector.tensor_add(
                                ye[0:sz, ci, dh * NB:(dh + 1) * NB],
                                p2s[ci * DH + dh][0:sz, :],
                                b2bc[0:sz, dh * NB:(dh + 1) * NB])

                    # scatter-add: acc += Sw^T @ Ye (dh pair shares lhsT);
                    # for the last expert, LayerNorm+store of tile t is
                    # emitted right after its scatter so the LN tail overlaps
                    # the remaining scatter matmuls.
                    n_sub = do // 512 if do % 512 == 0 and do > 512 else 1
                    for t in range(TT):
                        pss = [ps_tile(f"sc{dh}") for dh in range(DH)]
                        for ci, (co, sz) in enumerate(CC):
                            for dh in range(DH):
                                nc.tensor.matmul(
                                    pss[dh][:],
                                    sw_e[0:sz, ci, t * P:(t + 1) * P],
                                    ye[0:sz, ci, dh * NB:(dh + 1) * NB],
                                    start=(ci == 0), stop=(ci == len(CC) - 1),
                                )
                        for dh in range(DH):
                            a_sl = acc[:, t, dh * NB:(dh + 1) * NB]
                            nc.vector.tensor_add(a_sl, a_sl, pss[dh][:])
                        if e != e_ - 1:
                            continue
                        # ---- LayerNorm over do, then write out ----
                        a_t = acc[:, t, :]
                        st = lnp.tile([P, n_sub, 6], f32, tag="st")
                        a_view = a_t.rearrange("p (s d) -> p s d", s=n_sub)
                        for s in range(n_sub):
                            nc.vector.bn_stats(st[:, s, :], a_view[:, s, :])
                        mv = lnp.tile([P, 2], f32, tag="mv")
                        nc.vector.bn_aggr(mv[:], st[:])
                        rstd = lnp.tile([P, 1], f32, tag="rstd")
                        nc.scalar.activation(rstd[:], mv[:, 1:2], AF.Sqrt,
                                             bias=epst[:, 0:1], scale=1.0)
                        nc.vector.reciprocal(rstd[:], rstd[:])
                        o_t = outp.tile([P, do], f32, tag="o_t")
                        ob_t = outp.tile([P, do], bf16, tag="ob_t")
                        nc.vector.tensor_scalar_sub(o_t[:], a_t, mv[:, 0:1])
                        nc.vector.scalar_tensor_tensor(
                            o_t[:], in0=o_t[:], scalar=rstd[:], in1=gam_bc[:],
                            op0=ALU.mult, op1=ALU.mult,
                        )
                        nc.vector.tensor_add(ob_t[:], o_t[:], bet_bc[:])
                        nc.scalar.dma_start(out_t[t * P:(t + 1) * P, :], ob_t[:])

            if loop_n > 1:
                with tc.For_i(0, loop_n, 1):
                    _body()
            else:
                _body()

    nc.compile()
    return nc


def prep_inputs(x, W1, b1, W2, b2, gate_W, shared_W, shared_b, gamma, beta,
                n_cores=N_CORES, caps=CAPS_DEFAULT):
    """Host-side shard + relayout. Returns list of per-core in_maps."""
    n_tok = int(np.prod(x.shape[:-1]))
    d = x.shape[-1]
    e_, _, h_ = W1.shape
    do = W2.shape[-1]
    n_sh = n_tok // n_cores
    DC, HC, TT = d // P, h_ // P, n_sh // P

    HG = 4
    HGN = HC // HG
    DH = do // NB
    flat = np.ascontiguousarray(np.asarray(x, dtype=np.float32).reshape(n_tok, d))
    perm = _plan_routing(flat, np.asarray(gate_W, dtype=np.float32), caps,
                         n_cores)
    flat = np.ascontiguousarray(flat[perm])
    # weights: shared across cores (runtime copies per core)
    w1h = np.ascontiguousarray(
        np.asarray(W1, dtype=np.float32)
        .reshape(e_, DC, P, HGN, HG, P)
        .transpose(0, 3, 2, 4, 1, 5)                   # [e, g, p, j, dc, jj]
    ).astype(BF16)
    w2h = np.ascontiguousarray(
        np.asarray(W2, dtype=np.float32)
        .reshape(e_, HGN, HG, P, do)
        .transpose(0, 1, 3, 2, 4)                      # [e, g, p, j, do]
    ).astype(BF16)
    gwr = np.ascontiguousarray(
        np.asarray(gate_W, dtype=np.float32).reshape(DC, P, e_).transpose(1, 0, 2)
    )                                                  # [p, dc, e]
    swh = np.ascontiguousarray(
        np.asarray(shared_W, dtype=np.float32).reshape(DC, P, do).transpose(1, 0, 2)
    ).astype(BF16)                                     # [p, dc, do]
    b1h = np.ascontiguousarray(
        np.asarray(b1, dtype=np.float32).reshape(e_, HC, P).transpose(2, 0, 1)
    )                                                  # [p, e, hc]
    b2h = np.asarray(b2, dtype=np.float32).reshape(e_, do).astype(BF16)
    sbh = np.asarray(shared_b, dtype=np.float32).reshape(1, do).astype(BF16)
    gam = np.asarray(gamma, dtype=np.float32).reshape(do)
    bet = np.asarray(beta, dtype=np.float32).reshape(do)
    trih = np.triu(np.ones((P, P), np.float32)).astype(BF16)
    idnh = np.eye(P, dtype=np.float32).astype(BF16)
    ioth = np.arange(max(caps), dtype=np.float32)

    in_maps = []
    for c in range(n_cores):
        shard = flat[c * n_sh:(c + 1) * n_sh]          # [n_sh, d]
        xT = np.ascontiguousarray(
            shard.T.reshape(DC, P, TT, P).transpose(2, 1, 0, 3)
        )                                              # [t, p, dc, j]
        xtm = np.ascontiguousarray(
            shard.reshape(TT, P, d).transpose(1, 0, 2)
        ).astype(BF16)                                 # [p, tile, d]
        in_maps.append({
            "xT": xT, "xtm": xtm, "w1h": w1h, "w2h": w2h, "gwr": gwr,
            "swh": swh, "b1h": b1h, "b2h": b2h, "sbh": sbh, "gam": gam,
            "bet": bet, "tri": trih, "idn": idnh, "iot": ioth,
        })
    return in_maps, perm


_NC_CACHE = {}


def kernel(x, W1, b1, W2, b2, gate_W, shared_W, shared_b, gamma, beta):
    from concourse.bass_utils import run_bass_kernel_spmd

    n_tok = int(np.prod(x.shape[:-1]))
    n_sh = n_tok // N_CORES
    flat = np.asarray(x, dtype=np.float32).reshape(n_tok, x.shape[-1])
    caps = _expert_caps(flat, np.asarray(gate_W, dtype=np.float32))
    key = (n_sh, x.shape[-1], caps)
    if key not in _NC_CACHE:
        _NC_CACHE[key] = build_moe_bass(n_sh=n_sh, d=x.shape[-1],
                                        e_=W1.shape[0], h_=W1.shape[2],
                                        do=W2.shape[-1], caps=caps)
    nc = _NC_CACHE[key]
    in_maps, perm = prep_inputs(x, W1, b1, W2, b2, gate_W, shared_W, shared_b,
                                gamma, beta, caps=caps)
    res = run_bass_kernel_spmd(nc, in_maps, core_ids=list(range(N_CORES)))
    outs = [r["out"] for r in res.results]
    full = np.concatenate(outs, axis=0)               # [n_tok, do] (permuted)
    unperm = np.empty_like(full)
    unperm[perm] = full
    return unperm.reshape(*x.shape[:-1], full.shape[-1]).astype(np.float32)


# revision 52
# speedup vs baseline: 1.0788x; 1.0447x over previous
"""MoE layer (gate top-2 + 8 experts + shared expert + LayerNorm) on 8 TRN2 cores.

Strategy: data-parallel over tokens with SPARSE top-2 expert compute. The
host permutes tokens across the 8 cores (pure relayout, _plan_routing) so
per-(core,expert) load fits per-expert capacities caps[e] in {256, 288}
(288 for globally oversubscribed experts); the device still computes its own
routing. Per core (1024 tokens):

  1. Gate (fp32): logits -> top-2 masks; the sigmoid for the combine weights
     is deferred to after the gelu phase (avoids act-table thrash).
  2. Slot assignment ON DEVICE: per-expert token slots via an inclusive
     cumulative-sum of the expert mask (triangular-matrix matmul over the
     partition axis + sequential per-tile offsets).
  3. Dispatch: gather each expert's tokens with a matmul against the one-hot
     selection matrix SeT [tok, cap] (built by is_equal(iota, pos) * mask).
  4. Expert FFN on the gathered tokens only: GEMM1 and GEMM2 merged into one
     software-pipelined hc loop (G1(hc+1) is emitted before G2(hc) so the
     tensor engine never waits on the gelu); b2 is added during the Ye
     PSUM->SBUF copy from a broadcast tile.
  5. Combine: scatter-add weighted expert outputs back to token order with a
     matmul against Sw = SeT^T @ diag(combine_w)  (built on the tensor engine).
  6. Shared expert (dense, inside the gate loop) + LayerNorm; out is written
     bf16 and upcast on host.

This cuts tensor-engine work from ~140 GFLOP/core (dense all-expert) to
~60 GFLOP/core. Expert GEMMs run in bf16 with fp32 PSUM accumulation; the
gate runs fp32 (bf16 flips near-tie top-2 picks and fails the 2e-2 gate).
Tokens over capacity are dropped gracefully (one-hot never fires); the
planner leaves CAP_SLACK=4 headroom so in practice nothing drops.

Layout/perf notes (per core):
  - Weight streaming (128MB/iter) is split across BOTH HWDGE queues (W1 +
    x/out on the Activation queue via nc.scalar.dma_start, W2 on the SP
    queue): one queue saturates at ~213 GB/s, two reach ~400 GB/s.
  - Weight DMAs are batched 4 hc-tiles at a time (0.5-1MB per transfer);
    host pre-arranges W1/W2 so each transfer is contiguous.
  - GEMM1 (lhsT=W1 tile [128d,128h], rhs=XeT [128d, cap]) produces h in
    [h, slot] layout, exactly the lhsT layout GEMM2 needs.
  - Matmul moving operands are capped at 512 elements (ISA limit); GEMM2
    orders cc-outer/dh-inner so the dh pair reuses each stationary.
  - PSUM: 6 banks hold GEMM2's DH*CC accumulators across the merged loop;
    a 2-bank ring serves everything else.
"""

import numpy as np
import ml_dtypes

BF16 = ml_dtypes.bfloat16

# Problem shapes (hardcoded per contest contract).
B, S, D, E, H, DO = 4, 2048, 1024, 8, 4096, 1024
N_TOK = B * S
N_CORES = 8
P = 128
CAP = 320                 # per-expert token capacity per core (legacy uniform)
CAP_SLACK = 4             # host plans to CAP-CAP_SLACK kept tokens per (core,e)
NB = 512                  # PSUM bank width (fp32)
# Per-expert capacities for the balanced-permutation path: 288 for experts
# whose global top-2 count exceeds 8*(256-CAP_SLACK), else 256. The default
# matches the contest input (global counts [1971 1988 2045 2169 2147 1971
# 2012 2081]); kernel() recomputes from its actual input.
CAPS_DEFAULT = (256, 256, 288, 288, 288, 256, 256, 288)


def _expert_caps(flat, gate_W, n_cores=N_CORES, slack=CAP_SLACK):
    logits = flat @ gate_W
    srt = np.argsort(-logits, axis=1)
    G = np.bincount(srt[:, :2].ravel(), minlength=gate_W.shape[1])
    return tuple(288 if g > n_cores * (256 - slack) else 256 for g in G)


def _plan_routing(flat, gate_W, caps, n_cores=N_CORES, slack=CAP_SLACK):
    """Host-side token->core assignment balancing per-(core,expert) load.

    The device computes its own top-2 routing and assigns expert slots
    first-come-first-served in core token order with a fixed capacity of
    cap_dev per (core, expert). This planner permutes tokens across cores so
    every (core, expert) KEEP-count stays <= cap_dev - slack, and places any
    token whose expert contribution must be dropped (expert globally over
    8*(cap_dev-slack)) at the END of its core's order, choosing the
    lowest-combine-weight tokens as the drops. Pure relayout: the device
    still routes on its own; this only decides which tokens live on which
    core and in what order.
    """
    N, E_ = flat.shape[0], gate_W.shape[1]
    n_sh = N // n_cores
    cap_plan = np.asarray(caps, np.int64) - slack
    logits = flat @ gate_W
    srt = np.argsort(-logits, axis=1)
    e1, e2 = srt[:, 0], srt[:, 1]
    l1 = np.take_along_axis(logits, e1[:, None], 1)[:, 0]
    l2 = np.take_along_axis(logits, e2[:, None], 1)[:, 0]
    w2 = 1.0 / (1.0 + np.exp(np.minimum(l1 - l2, 50.0)))
    wts = np.stack([1.0 - w2, w2], 1)                  # [N, 2]
    exps = np.stack([e1, e2], 1)                       # [N, 2]

    # greedy balance: every token-expert pair counts toward its core's load
    cnt = np.zeros((n_cores, E_), np.int64)
    size = np.zeros(n_cores, np.int64)
    assign = np.full(N, -1, np.int64)
    rng = np.random.default_rng(0)
    for t in rng.permutation(N):
        ea, eb = int(exps[t, 0]), int(exps[t, 1])
        best, bkey = -1, None
        for c in range(n_cores):
            if size[c] >= n_sh:
                continue
            over = ((cnt[c, ea] >= cap_plan[ea]) +
                    (cnt[c, eb] >= cap_plan[eb]))
            load = max(int(cnt[c, ea]) - int(cap_plan[ea]),
                       int(cnt[c, eb]) - int(cap_plan[eb]))
            key = (over, load, int(size[c]))
            if bkey is None or key < bkey:
                best, bkey = c, key
        assign[t] = best
        size[best] += 1
        cnt[best, ea] += 1
        cnt[best, eb] += 1

    # per-core ordering: tokens carrying a low-weight contribution of an
    # over-capacity (core, expert) go last, so the device's FCFS slotting
    # drops exactly those.
    perm_parts = []
    caps_a = np.asarray(caps, np.int64)
    for c in range(n_cores):
        toks = np.where(assign == c)[0]
        over_e = set(np.where(cnt[c] > caps_a)[0].tolist())
        key = np.full(len(toks), np.inf)
        for k in range(2):
            sel = np.array([int(exps[t, k]) in over_e for t in toks])
            if sel.any():
                key[sel] = np.minimum(key[sel], wts[toks[sel], k])
        perm_parts.append(toks[np.argsort(-key, kind="stable")])
    return np.concatenate(perm_parts)


def _chunks(c):
    out = []
    o = 0
    while o < c:
        out.append((o, min(P, c - o)))
        o += P
    return out


def build_moe_bass(n_sh=N_TOK // N_CORES, d=D, e_=E, h_=H, do=DO, loop_n=1,
                   caps=CAPS_DEFAULT):
    """Build the single-core SPMD Bass program.

    loop_n > 1 wraps the whole compute body in an on-device For_i loop —
    used only for timing (amortizes the host dispatch overhead away).
    """
    import concourse.bass as bass
    import concourse.mybir as mybir
    import concourse.tile as tile
    from concourse import bacc

    f32 = mybir.dt.float32
    bf16 = mybir.dt.bfloat16
    AF = mybir.ActivationFunctionType
    ALU = mybir.AluOpType
    X = mybir.AxisListType.X

    DC = d // P        # d chunks
    HC = h_ // P       # h chunks
    TT = n_sh // P     # token tiles of 128
    DH = do // NB      # do chunks of 512
    CAPM = max(caps)   # padded tile capacity
    CCs = [_chunks(c) for c in caps]   # per-expert capacity chunks
    CCM = max(len(cc) for cc in CCs)

    HG = 4             # hc tiles per weight-DMA group
    HGN = HC // HG

    nc = bacc.Bacc("TRN2", target_bir_lowering=False)

    xT_t = nc.dram_tensor("xT", [TT, P, DC, P], f32, kind="ExternalInput")
    xtm_t = nc.dram_tensor("xtm", [P, TT, d], bf16, kind="ExternalInput")
    w1_t = nc.dram_tensor("w1h", [e_, HGN, P, HG, DC, P], bf16,
                          kind="ExternalInput")
    w2_t = nc.dram_tensor("w2h", [e_, HGN, P, HG, do], bf16,
                          kind="ExternalInput")
    gw_t = nc.dram_tensor("gwr", [P, DC, e_], f32, kind="ExternalInput")
    sw_t = nc.dram_tensor("swh", [P, DC, do], bf16, kind="ExternalInput")
    b1_t = nc.dram_tensor("b1h", [P, e_, HC], f32, kind="ExternalInput")
    b2_t = nc.dram_tensor("b2h", [e_, do], bf16, kind="ExternalInput")
    sb_t = nc.dram_tensor("sbh", [1, do], bf16, kind="ExternalInput")
    gam_t = nc.dram_tensor("gam", [do], f32, kind="ExternalInput")
    bet_t = nc.dram_tensor("bet", [do], f32, kind="ExternalInput")
    tri_t = nc.dram_tensor("tri", [P, P], bf16, kind="ExternalInput")
    idn_t = nc.dram_tensor("idn", [P, P], bf16, kind="ExternalInput")
    iot_t = nc.dram_tensor("iot", [CAPM], f32, kind="ExternalInput")
    out_t = nc.dram_tensor("out", [n_sh, do], bf16, kind="ExternalOutput")

    with tile.TileContext(nc) as tc:
        with (
            tc.tile_pool(name="resident", bufs=1) as resident,
            tc.tile_pool(name="xtp", bufs=2) as xtp,
            tc.tile_pool(name="xbp", bufs=2) as xbp,
            tc.tile_pool(name="setp", bufs=2) as setp,
            tc.tile_pool(name="xep", bufs=2) as xep,
            tc.tile_pool(name="hp", bufs=4) as hp,
            tc.tile_pool(name="yep", bufs=2) as yep,
            tc.tile_pool(name="swp", bufs=2) as swp,
            tc.tile_pool(name="dgp", bufs=2) as dgp,
            tc.tile_pool(name="b2p", bufs=2) as b2p,
            tc.tile_pool(name="w1p", bufs=3) as w1p,
            tc.tile_pool(name="w2p", bufs=3) as w2p,
            tc.tile_pool(name="gatep", bufs=2) as gatep,
            tc.tile_pool(name="lnp", bufs=3) as lnp,
            tc.tile_pool(name="outp", bufs=2) as outp,
            tc.tile_pool(name="psp", bufs=2, space="PSUM") as psp,
            tc.tile_pool(name="psB", bufs=DH * CCM, space="PSUM") as psB,
        ):
            def ps_tile(name):
                return psp.tile([P, NB], f32, tag="ps", name=name)

            # ---- resident loads ----
            gw = resident.tile([P, DC, e_], f32)
            nc.sync.dma_start(gw[:], gw_t[:])
            sw = resident.tile([P, DC, do], bf16)
            nc.sync.dma_start(sw[:], sw_t[:])
            b1 = resident.tile([P, e_, HC], f32)
            nc.sync.dma_start(b1[:], b1_t[:])
            sb = resident.tile([1, do], bf16)
            nc.sync.dma_start(sb[:], sb_t[:])
            tri = resident.tile([P, P], bf16)
            nc.sync.dma_start(tri[:], tri_t[:])
            idn = resident.tile([P, P], bf16)
            nc.sync.dma_start(idn[:], idn_t[:])
            xtm = resident.tile([P, TT, d], bf16)
            nc.scalar.dma_start(xtm[:], xtm_t[:])

            ones = resident.tile([1, P], bf16)
            nc.vector.memset(ones[:], 1.0)
            onef = resident.tile([1, P], f32)
            nc.vector.memset(onef[:], 1.0)
            onec = resident.tile([P, 1], bf16)
            nc.vector.memset(onec[:], 1.0)
            epst = resident.tile([P, 1], f32)
            nc.vector.memset(epst[:], 1e-5)

            # gamma/beta/iota broadcast across partitions via stride-0 DMA
            def bc_load(dst, src_ap):
                nc.gpsimd.dma_start(
                    out=dst,
                    in_=bass.AP(tensor=src_ap.tensor, offset=src_ap.offset,
                                ap=[[0, P]] + [list(a) for a in src_ap.ap]),
                )

            gam_bc = resident.tile([P, do], f32)
            bc_load(gam_bc[:], gam_t[:])
            bet_bc = resident.tile([P, do], f32)
            bc_load(bet_bc[:], bet_t[:])
            iota_bc = resident.tile([P, CAPM], f32)
            bc_load(iota_bc[:], iot_t[:])

            # accumulator [tok, do] fp32; combine weights / masks [tok, e]
            acc = resident.tile([P, TT, do], f32)
            comb = resident.tile([P, TT, e_], f32)
            mskf = resident.tile([P, TT, e_], f32)
            mskb = resident.tile([P, TT, e_], bf16)
            msk1a = resident.tile([P, TT, e_], f32)
            msk2a = resident.tile([P, TT, e_], f32)
            d21a = resident.tile([P, TT], f32)
            w2va = resident.tile([P, TT], f32)
            w1va = resident.tile([P, TT], f32)
            pos = resident.tile([P, TT, e_], f32)
            offr = resident.tile([1, TT, e_], f32)
            totr = resident.tile([1, TT, e_], f32)
            cum = resident.tile([P, TT, e_], f32)

            def _body():
                # ---- gate (fp32) + shared expert, one token-tile at a time --
                for t in range(TT):
                    xt_c = xtp.tile([P, DC, P], f32, tag="xt_c")
                    nc.sync.dma_start(xt_c[:], xT_t[t])
                    xb_c = xbp.tile([P, DC, P], bf16, tag="xb_c")
                    nc.vector.tensor_copy(xb_c[:], xt_c[:])
                    pg = ps_tile("pg")
                    for dc in range(DC):
                        nc.tensor.matmul(
                            pg[:, 0:e_], xt_c[:, dc, :], gw[:, dc, :],
                            start=(dc == 0), stop=(dc == DC - 1),
                        )
                    lg = gatep.tile([P, e_], f32, tag="lg")
                    nc.vector.tensor_copy(lg[:], pg[:, 0:e_])
                    m1 = gatep.tile([P, 1], f32, tag="m1")
                    nc.vector.reduce_max(m1[:], lg[:], axis=X)
                    mask1 = gatep.tile([P, e_], f32, tag="mask1")
                    nc.vector.tensor_scalar(mask1[:], lg[:], m1[:], None, ALU.is_ge)
                    l2 = gatep.tile([P, e_], f32, tag="l2")
                    nc.vector.scalar_tensor_tensor(
                        l2[:], in0=mask1[:], scalar=-1e30, in1=lg[:],
                        op0=ALU.mult, op1=ALU.add,
                    )
                    m2 = gatep.tile([P, 1], f32, tag="m2")
                    nc.vector.reduce_max(m2[:], l2[:], axis=X)
                    mask2 = gatep.tile([P, e_], f32, tag="mask2")
                    nc.vector.tensor_scalar(mask2[:], l2[:], m2[:], None, ALU.is_ge)
                    nc.vector.tensor_add(mskf[:, t, :], mask1[:], mask2[:])
                    nc.vector.tensor_copy(mskb[:, t, :], mskf[:, t, :])
                    nc.vector.tensor_sub(d21a[:, t:t + 1], m2[:], m1[:])
                    nc.vector.tensor_copy(msk1a[:, t, :], mask1[:])
                    nc.vector.tensor_copy(msk2a[:, t, :], mask2[:])

                    # shared expert: acc = 0.5*gelu(x @ shared_W + shared_b)
                    for dh in range(DH):
                        ps = ps_tile("sh")
                        nc.tensor.matmul(
                            ps[:], ones[0:1, :], sb[0:1, dh * NB:(dh + 1) * NB],
                            start=True, stop=False,
                        )
                        for dc in range(DC):
                            nc.tensor.matmul(
                                ps[:], xb_c[:, dc, :],
                                sw[:, dc, dh * NB:(dh + 1) * NB],
                                start=False, stop=(dc == DC - 1),
                            )
                        a_sl = acc[:, t, dh * NB:(dh + 1) * NB]
                        nc.scalar.activation(a_sl, ps[:], AF.Gelu)
                        nc.vector.tensor_scalar_mul(a_sl, a_sl, 0.5)

                # deferred gate weights: one sigmoid for all tiles (avoids
                # act-table thrash between Gelu and Sigmoid inside the loop)
                nc.scalar.activation(w2va[:], d21a[:], AF.Sigmoid)
                nc.vector.tensor_scalar(w1va[:], w2va[:], -1.0, 1.0,
                                        ALU.mult, ALU.add)
                for t in range(TT):
                    nc.vector.tensor_scalar_mul(msk1a[:, t, :], msk1a[:, t, :],
                                                w1va[:, t:t + 1])
                    nc.vector.scalar_tensor_tensor(
                        comb[:, t, :], in0=msk2a[:, t, :],
                        scalar=w2va[:, t:t + 1], in1=msk1a[:, t, :],
                        op0=ALU.mult, op1=ALU.add,
                    )

                # ---- slot assignment: pos = incl-cumsum(mask) - 1 ----------
                pc = ps_tile("pc")
                nc.tensor.matmul(pc[:, 0:TT * e_], tri[:], mskb[:],
                                 start=True, stop=True)
                nc.vector.tensor_copy(cum[:], pc[:, 0:TT * e_])
                pt = ps_tile("pt")
                nc.tensor.matmul(pt[0:1, 0:TT * e_], onec[:], mskb[:],
                                 start=True, stop=True)
                nc.vector.tensor_copy(totr[:], pt[0:1, 0:TT * e_])
                nc.vector.memset(offr[0:1, 0, :], 0.0)
                for t in range(1, TT):
                    nc.vector.tensor_add(offr[0:1, t, :], offr[0:1, t - 1, :],
                                         totr[0:1, t - 1, :])
                pb = ps_tile("pb")
                nc.tensor.matmul(pb[:, 0:TT * e_], onef[0:1, :], offr[:],
                                 start=True, stop=True)
                # pos = (cum - 1) + off_broadcast
                nc.vector.scalar_tensor_tensor(
                    pos[:], in0=cum[:], scalar=-1.0, in1=pb[:, 0:TT * e_],
                    op0=ALU.add, op1=ALU.add,
                )

                # ---- experts (sparse, capacity CAP) ------------------------
                for e in range(e_):
                    cap, CC = caps[e], CCs[e]
                    # SeT [tok, cap] one-hot, bf16
                    seT = setp.tile([P, TT, CAPM], bf16, tag="seT")
                    for t in range(TT):
                        nc.vector.tensor_scalar(seT[:, t, 0:cap],
                                                iota_bc[:, 0:cap],
                                                pos[:, t, e:e + 1],
                                                mskf[:, t, e:e + 1],
                                                ALU.is_equal, ALU.mult)

                    # dispatch: XeT[d, c] = sum_t x[t, d] * SeT[t, c]
                    xeT = xep.tile([P, DC, CAPM], bf16, tag="xeT")
                    for dc in range(DC):
                        pd = ps_tile("pd")
                        for tt in range(TT):
                            nc.tensor.matmul(
                                pd[:, 0:cap], xtm[:, tt, dc * P:(dc + 1) * P],
                                seT[:, tt, 0:cap],
                                start=(tt == 0), stop=(tt == TT - 1),
                            )
                        nc.vector.tensor_copy(xeT[:, dc, 0:cap], pd[:, 0:cap])

                    # Sw = SeT^T @ diag(comb_e)  [CAP, tok] bf16
                    sw_e = swp.tile([P, CCM, n_sh], bf16, tag="sw_e")
                    for t in range(TT):
                        dg = dgp.tile([P, P], bf16, tag="dg")
                        nc.vector.tensor_scalar_mul(dg[:], idn[:],
                                                    comb[:, t, e:e + 1])
                        for ci, (co, sz) in enumerate(CC):
                            pw = ps_tile("pw")
                            nc.tensor.matmul(pw[0:sz, 0:P],
                                             seT[:, t, co:co + sz], dg[:],
                                             start=True, stop=True)
                            nc.vector.tensor_copy(
                                sw_e[0:sz, ci, t * P:(t + 1) * P],
                                pw[0:sz, 0:P])

                    # GEMM1+GEMM2 merged, software-pipelined per hc:
                    #   G1(hc) -> gelu(hc) -> (next iter: G1(hc+1) emitted
                    #   before G2(hc) so PE never waits on the gelu).
                    # GEMM2's DH*CC psum tiles accumulate across all hc.
                    b2bc = b2p.tile([P, do], bf16, tag="b2bc")
                    bc_load(b2bc[:], b2_t[e])
                    ye = yep.tile([P, CCM, do], bf16, tag="ye")
                    p2s = [psB.tile([P, NB], f32, tag="p2", name=f"p2_{i}")
                           for i in range(DH * len(CC))]
                    h_tiles = {}
                    w2_tiles = {}

                    def emit_g1(hc):
                        g, j = hc // HG, hc % HG
                        if j == 0:
                            # W1 kicks ride the Activation HWDGE queue, W2 the
                            # SP queue: two queues ~doubles achieved HBM BW.
                            w1t = w1p.tile([P, HG, DC, P], bf16, tag="w1t")
                            nc.scalar.dma_start(w1t[:], w1_t[e, g])
                            emit_g1.w1t = w1t
                            w2t = w2p.tile([P, HG, do], bf16, tag="w2t")
                            nc.sync.dma_start(w2t[:], w2_t[e, g])
                            w2_tiles[g] = w2t
                        p1 = ps_tile("p1")
                        for dc in range(DC):
                            nc.tensor.matmul(
                                p1[:, 0:cap], emit_g1.w1t[:, j, dc, :],
                                xeT[:, dc, 0:cap],
                                start=(dc == 0), stop=(dc == DC - 1),
                            )
                        h_t = hp.tile([P, CAPM], bf16, tag="h_t")
                        nc.scalar.activation(
                            h_t[:, 0:cap], p1[:, 0:cap],
                            AF.Gelu, bias=b1[:, e, hc:hc + 1], scale=1.0,
                        )
                        h_tiles[hc] = h_t

                    def emit_g2(hc):
                        g, j = hc // HG, hc % HG
                        h_t = h_tiles.pop(hc)
                        w2t = w2_tiles[g]
                        # cc outer, dh inner: the dh pair shares one stationary
                        for ci, (co, sz) in enumerate(CC):
                            for dh in range(DH):
                                nc.tensor.matmul(
                                    p2s[ci * DH + dh][0:sz, :],
                                    h_t[0:P, co:co + sz],
                                    w2t[:, j, dh * NB:(dh + 1) * NB],
                                    start=(hc == 0), stop=(hc == HC - 1),
                                )

                    emit_g1(0)
                    for hc in range(1, HC):
                        emit_g1(hc)
                        emit_g2(hc - 1)
                    emit_g2(HC - 1)
                    for ci, (co, sz) in enumerate(CC):
                        for dh in range(DH):
                            nc.vector.tensor_add(
                                ye[0:sz, ci, dh * NB:(dh + 1) * NB],
                                p2s[ci * DH + dh][0:sz, :],
                                b2bc[0:sz, dh * NB:(dh + 1) * NB])

                    # scatter-add: acc += Sw^T @ Ye (dh pair shares lhsT);
                    # for the last expert, LayerNorm+store of tile t is
                    # emitted right after its scatter so the LN tail overlaps
                    # the remaining scatter matmuls.
                    n_sub = do // 512 if do % 512 == 0 and do > 512 else 1
                    for t in range(TT):
                        pss = [ps_tile(f"sc{dh}") for dh in range(DH)]
                        for ci, (co, sz) in enumerate(CC):
                            for dh in range(DH):
                                nc.tensor.matmul(
                                    pss[dh][:],
                                    sw_e[0:sz, ci, t * P:(t + 1) * P],
                                    ye[0:sz, ci, dh * NB:(dh + 1) * NB],
                                    start=(ci == 0), stop=(ci == len(CC) - 1),
                                )
                        for dh in range(DH):
                            a_sl = acc[:, t, dh * NB:(dh + 1) * NB]
                            nc.vector.tensor_add(a_sl, a_sl, pss[dh][:])
                        if e != e_ - 1:
                            continue
                        # ---- LayerNorm over do, then write out ----
                        a_t = acc[:, t, :]
                        st = lnp.tile([P, n_sub, 6], f32, tag="st")
                        a_view = a_t.rearrange("p (s d) -> p s d", s=n_sub)
                        for s in range(n_sub):
                            nc.vector.bn_stats(st[:, s, :], a_view[:, s, :])
                        mv = lnp.tile([P, 2], f32, tag="mv")
                        nc.vector.bn_aggr(mv[:], st[:])
                        rstd = lnp.tile([P, 1], f32, tag="rstd")
                        nc.scalar.activation(rstd[:], mv[:, 1:2], AF.Sqrt,
                                             bias=epst[:, 0:1], scale=1.0)
                        nc.vector.reciprocal(rstd[:], rstd[:])
                        o_t = outp.tile([P, do], f32, tag="o_t")
                        ob_t = outp.tile([P, do], bf16, tag="ob_t")
                        nc.vector.tensor_scalar_sub(o_t[:], a_t, mv[:, 0:1])
                        nc.vector.scalar_tensor_tensor(
                            o_t[:], in0=o_t[:], scalar=rstd[:], in1=gam_bc[:],
                            op0=ALU.mult, op1=ALU.mult,
                        )
                        nc.vector.tensor_add(ob_t[:], o_t[:], bet_bc[:])
                        nc.scalar.dma_start(out_t[t * P:(t + 1) * P, :], ob_t[:])

            if loop_n > 1:
                with tc.For_i(0, loop_n, 1):
                    _body()
            else:
                _body()

    nc.compile()
    return nc


def prep_inputs(x, W1, b1, W2, b2, gate_W, shared_W, shared_b, gamma, beta,
                n_cores=N_CORES, caps=CAPS_DEFAULT):
    """Host-side shard + relayout. Returns list of per-core in_maps."""
    n_tok = int(np.prod(x.shape[:-1]))
    d = x.shape[-1]
    e_, _, h_ = W1.shape
    do = W2.shape[-1]
    n_sh = n_tok // n_cores
    DC, HC, TT = d // P, h_ // P, n_sh // P

    HG = 4
    HGN = HC // HG
    DH = do // NB
    flat = np.ascontiguousarray(np.asarray(x, dtype=np.float32).reshape(n_tok, d))
    perm = _plan_routing(flat, np.asarray(gate_W, dtype=np.float32), caps,
                         n_cores)
    flat = np.ascontiguousarray(flat[perm])
    # weights: shared across cores (runtime copies per core)
    w1h = np.ascontiguousarray(
        np.asarray(W1, dtype=np.float32)
        .reshape(e_, DC, P, HGN, HG, P)
        .transpose(0, 3, 2, 4, 1, 5)                   # [e, g, p, j, dc, jj]
    ).astype(BF16)
    w2h = np.ascontiguousarray(
        np.asarray(W2, dtype=np.float32)
        .reshape(e_, HGN, HG, P, do)
        .transpose(0, 1, 3, 2, 4)                      # [e, g, p, j, do]
    ).astype(BF16)
    gwr = np.ascontiguousarray(
        np.asarray(gate_W, dtype=np.float32).reshape(DC, P, e_).transpose(1, 0, 2)
    )                                                  # [p, dc, e]
    swh = np.ascontiguousarray(
        np.asarray(shared_W, dtype=np.float32).reshape(DC, P, do).transpose(1, 0, 2)
    ).astype(BF16)                                     # [p, dc, do]
    b1h = np.ascontiguousarray(
        np.asarray(b1, dtype=np.float32).reshape(e_, HC, P).transpose(2, 0, 1)
    )                                                  # [p, e, hc]
    b2h = np.asarray(b2, dtype=np.float32).reshape(e_, do).astype(BF16)
    sbh = np.asarray(shared_b, dtype=np.float32).reshape(1, do).astype(BF16)
    gam = np.asarray(gamma, dtype=np.float32).reshape(do)
    bet = np.asarray(beta, dtype=np.float32).reshape(do)
    trih = np.triu(np.ones((P, P), np.float32)).astype(BF16)
    idnh = np.eye(P, dtype=np.float32).astype(BF16)
    ioth = np.arange(max(caps), dtype=np.float32)

    in_maps = []
    for c in range(n_cores):
        shard = flat[c * n_sh:(c + 1) * n_sh]          # [n_sh, d]
        xT = np.ascontiguousarray(
            shard.T.reshape(DC, P, TT, P).transpose(2, 1, 0, 3)
        )                                              # [t, p, dc, j]
        xtm = np.ascontiguousarray(
            shard.reshape(TT, P, d).transpose(1, 0, 2)
        ).astype(BF16)                                 # [p, tile, d]
        in_maps.append({
            "xT": xT, "xtm": xtm, "w1h": w1h, "w2h": w2h, "gwr": gwr,
            "swh": swh, "b1h": b1h, "b2h": b2h, "sbh": sbh, "gam": gam,
            "bet": bet, "tri": trih, "idn": idnh, "iot": ioth,
        })
    return in_maps, perm


_NC_CACHE = {}


def kernel(x, W1, b1, W2, b2, gate_W, shared_W, shared_b, gamma, beta):
    from concourse.bass_utils import run_bass_kernel_spmd

    n_tok = int(np.prod(x.shape[:-1]))
    n_sh = n_tok // N_CORES
    flat = np.asarray(x, dtype=np.float32).reshape(n_tok, x.shape[-1])
    caps = _expert_caps(flat, np.asarray(gate_W, dtype=np.float32))
    key = (n_sh, x.shape[-1], caps)
    if key not in _NC_CACHE:
        _NC_CACHE[key] = build_moe_bass(n_sh=n_sh, d=x.shape[-1],
                                        e_=W1.shape[0], h_=W1.shape[2],
                                        do=W2.shape[-1], caps=caps)
    nc = _NC_CACHE[key]
    in_maps, perm = prep_inputs(x, W1, b1, W2, b2, gate_W, shared_W, shared_b,
                                gamma, beta, caps=caps)
    res = run_bass_kernel_spmd(nc, in_maps, core_ids=list(range(N_CORES)))
    outs = [r["out"] for r in res.results]
    full = np.concatenate(outs, axis=0)               # [n_tok, do] (permuted)
    unperm = np.empty_like(full)
    unperm[perm] = full
    return unperm.reshape(*x.shape[:-1], full.shape[-1]).astype(np.float32)
